# revision 24
# baseline (speedup 1.0000x reference)
"""LEM cell (ODE2) Bass kernel for Trainium2, 8-core data-parallel, fp8 PE.

Math (per batch row b):
  ti = x @ W_ih.T + b_ih                  # [B, 4H]
  th = y @ W_hh.T + b_hh                  # [B, 3H]
  tdt = dt @ W_dt.T + b_dt                # [B, 2]
  ms_dt_bar = sig(tdt[:,0]) * sig(ti[:, :H]   + th[:, :H])
  ms_dt     = sig(tdt[:,1]) * sig(ti[:, H:2H] + th[:, H:2H])
  z_new = (1-ms_dt) * z + ms_dt * tanh(ti[:, 3H:] + th[:, 2H:3H])
  y_new = (1-ms_dt_bar) * y + ms_dt_bar * tanh(z_new @ W_z.T + b_z + i_z)
  returns (y_new, z_new)

Strategy: shard batch across 8 cores (2048 rows each). All on-chip tensors are
feature-major ([feature_tile=128 partitions, batch columns free]). The GEMMs
run as fp8e4 (e4m3) with the DoubleRow perf mode: each matmul contracts 256
features (two 128-tiles packed in the AP's dim1) at the 157 TF/s fp8 peak —
2x the fp32r rate of the fp32 baseline. Host pre-scales x/y by 16 and weights
by 4096 (powers of two; undone exactly by the activation scale 1/65536);
z_new is cast on-chip to fp8*16 for the third GEMM. The i+h / i_z+z@Wz sums
come free by accumulating both halves of each packed weight block into the
same PSUM bank. The whole pointwise pipeline (gates, blends, outputs) runs in
fp16 — the fp8 GEMM quantization error dominates, so fp16 rounding is
invisible (simulated end-to-end rel err 1.40e-2, gate 2e-2) — which halves
DVE/Pool SBUF traffic and the z/y/output DMA volume. The per-column dt-gate
products m = sig(tdt)*sig(gate) are computed while the second GEMM of each
feature tile streams, so only a 3-op chain (sub, mul, add) trails the last
matmul of a tile. The last feature tiles run at quarter granularity to
shorten the drain tail.
"""

import os
import sys

_REPO = "/opt/trn_rl_repo"
if _REPO not in sys.path:
    sys.path.insert(0, _REPO)

from contextlib import ExitStack

import numpy as np
import ml_dtypes

import concourse.bacc as bacc
import concourse.bass as bass
import concourse.tile as tile
from concourse import mybir
from concourse.bass_utils import run_bass_kernel_spmd

P = 128
F32 = mybir.dt.float32
F16 = mybir.dt.float16
F8 = mybir.dt.float8e4
E4 = ml_dtypes.float8_e4m3
AF = mybir.ActivationFunctionType
DR = mybir.MatmulPerfMode.DoubleRow

N_CORES = 8
H = 1024
K = 1024
NKT = K // P  # 8 contraction tiles per 1024-dim operand
NJT = H // P  # 8 output feature tiles
SA = 16.0     # activation fp8 pre-scale
SW = 4096.0   # weight fp8 pre-scale
INV = 1.0 / (SA * SW)

LAST_RESULTS = None  # BassKernelResults of the most recent kernel() call


def build_nc(Bs, wdt00, wdt10, ch=512, half=1024):
    NCH = Bs // ch    # PSUM banks per gemm block
    NH = Bs // half   # pointwise sub-tiles per jt

    nc = bacc.Bacc(trn_type="TRN2", target_bir_lowering=False)

    x8d = nc.declare_dram_parameter("x8", [P, NKT, Bs], F8, isOutput=False)
    y8d = nc.declare_dram_parameter("y8", [P, NKT, Bs], F8, isOutput=False)
    z16d = nc.declare_dram_parameter("z16", [P, NJT, Bs], F16, isOutput=False)
    y16d = nc.declare_dram_parameter("y16", [P, NJT, Bs], F16, isOutput=False)
    dtrd = nc.declare_dram_parameter("dtr", [1, Bs], F32, isOutput=False)
    Wd = {
        g: nc.declare_dram_parameter(g, [NJT, P, 2 * NKT, P], F8, isOutput=False)
        for g in ("wd2", "wy", "wd1", "wg3")
    }
    biasd = nc.declare_dram_parameter("biasP", [P, 4 * NJT + 2], F32, isOutput=False)
    zod = nc.declare_dram_parameter("z_newP", [P, NJT, Bs], F16, isOutput=True)
    yod = nc.declare_dram_parameter("y_newP", [P, NJT, Bs], F16, isOutput=True)

    with tile.TileContext(nc) as tc, ExitStack() as ctx:
        cpool = ctx.enter_context(tc.tile_pool(name="cpool", bufs=1))
        wstr = ctx.enter_context(tc.tile_pool(name="wstr", bufs=2))
        pspool = ctx.enter_context(tc.tile_pool(name="pspool", bufs=8, space="PSUM"))
        apool = ctx.enter_context(tc.tile_pool(name="apool", bufs=3))
        zpool = ctx.enter_context(tc.tile_pool(name="zpool", bufs=2))
        dpool = ctx.enter_context(tc.tile_pool(name="dpool", bufs=3))
        opool = ctx.enter_context(tc.tile_pool(name="opool", bufs=2))

        bias_sb = cpool.tile([P, 4 * NJT + 2], F32, name="bias_sb")
        dt_sb = cpool.tile([1, Bs], F32, name="dt_sb")

        def bcol(g, jt):
            i = g * NJT + jt
            return bias_sb[:, i : i + 1]

        # jt0/jt1 weights lead the scalar queue, halved so the x-contraction
        # half (used by the first 4 pair-steps) lands before the y half.
        wstream = {"wd2": {}, "wy": {}, "wd1": {}, "wg3": {}}

        def stream_w(g, jt, eng=None, split=False):
            t_ = wstr.tile([P, 2 * NKT, P], F8, name=g, tag=g)
            eng = eng or nc.sync
            if split:
                eng.dma_start(t_[:, 0:NKT, :], Wd[g][jt][:, 0:NKT, :])
                eng.dma_start(t_[:, NKT : 2 * NKT, :], Wd[g][jt][:, NKT : 2 * NKT, :])
            else:
                eng.dma_start(t_[:], Wd[g][jt])
            wstream[g][jt] = t_

        stream_w("wd2", 0, nc.scalar, split=True)
        stream_w("wy", 0, nc.scalar, split=True)

        # fp8 activations, resident for the whole kernel. x on sync, y on
        # gpsimd: separate queues so both stream from t=0, issue-ordered by
        # first PE use; column-halved so the first chunks land early during
        # the 8-core cold-start HBM burst.
        x8_sb = cpool.tile([P, NKT, Bs], F8, name="x8_sb")
        y8_sb = cpool.tile([P, NKT, Bs], F8, name="y8_sb")
        hb = Bs // 2
        for t in range(NKT // 2):
            for h in range(2):
                hs = slice(h * hb, (h + 1) * hb)
                eng = nc.sync
                eng.dma_start(x8_sb[:, 2 * t : 2 * t + 2, hs], x8d[:, 2 * t : 2 * t + 2, hs])
        for t in range(NKT // 2):
            for h in range(2):
                hs = slice(h * hb, (h + 1) * hb)
                eng = nc.gpsimd if h == 0 else nc.scalar
                eng.dma_start(y8_sb[:, 2 * t : 2 * t + 2, hs], y8d[:, 2 * t : 2 * t + 2, hs])
        zn8_sb = cpool.tile([P, NJT, Bs], F8, name="zn8_sb")

        nc.sync.dma_start(dt_sb[:], dtrd[0:1, :])
        nc.sync.dma_start(bias_sb[:], biasd[:, :])
        stream_w("wd2", 1, nc.scalar)
        stream_w("wy", 1, nc.scalar)

        # per-batch dt gates (fp16 broadcast tiles). The broadcasts are
        # emitted after the z16(0) prefetch: their sg-semaphore waits would
        # otherwise block the gpsimd queue head and delay that transfer.
        bc1 = cpool.tile([P, Bs], F16, name="bc1")
        bc2 = cpool.tile([P, Bs], F16, name="bc2")
        nc.scalar.activation(
            bc1[0:1, :], dt_sb[:], AF.Sigmoid,
            bias=bias_sb[0:1, 4 * NJT : 4 * NJT + 1], scale=wdt00,
        )
        nc.scalar.activation(
            bc2[0:1, :], dt_sb[:], AF.Sigmoid,
            bias=bias_sb[0:1, 4 * NJT + 1 : 4 * NJT + 2], scale=wdt10,
        )

        # streamed fp16 blend operands (z for phase B, y for phase C)
        zt_tiles, yt_tiles = {}, {}

        def prefetch_z(jt):
            zt = zpool.tile([P, Bs], F16, name="zt", tag="zt")
            nc.gpsimd.dma_start(zt[:], z16d[:, jt, :])
            zt_tiles[jt] = zt

        def prefetch_y(jt):
            yt = zpool.tile([P, Bs], F16, name="yt", tag="yt")
            nc.gpsimd.dma_start(yt[:], y16d[:, jt, :])
            yt_tiles[jt] = yt

        def gemm_block(w_sb, rhs_a, rhs_b, c_outer=False):
            """2048-deep contraction into NCH psum banks via DoubleRow pairs.

            Weight block dim1 holds 16 ktiles: [0:8] pair with rhs_a,
            [8:16] with rhs_b. c-inner keeps each weight pair loaded in the
            PE array for NCH consecutive matmuls; c_outer instead finishes
            each bank as early as possible (4x the weight loads — used only
            for the final block so its drain chain overlaps the gemm).
            """
            banks = [pspool.tile([P, ch], F32, name="ps", tag="ps") for _ in range(NCH)]

            def mm(t, c):
                if t < NKT // 2:
                    w_ap = w_sb[:, 2 * t : 2 * t + 2, :]
                    r_src, r0 = rhs_a, 2 * t
                else:
                    tb = t - NKT // 2
                    w_ap = w_sb[:, NKT + 2 * tb : NKT + 2 * tb + 2, :]
                    r_src, r0 = rhs_b, 2 * tb
                nc.tensor.matmul(
                    banks[c][:],
                    lhsT=w_ap,
                    rhs=r_src[:, r0 : r0 + 2, c * ch : (c + 1) * ch],
                    start=(t == 0),
                    stop=(t == NKT - 1),
                    perf_mode=DR,
                )

            if c_outer:
                for c in range(NCH):
                    for t in range(NKT):
                        mm(t, c)
            else:
                for t in range(NKT):
                    for c in range(NCH):
                        mm(t, c)
            return banks

        def evict(banks, dst, q, gran, func, fbias):
            """ACT-evict psum bank range q*gran..(q+1)*gran into dst[0:gran]."""
            done = 0
            while done < gran:
                g0 = q * gran + done
                b, off = g0 // ch, g0 % ch
                n = min(ch - off, gran - done)
                nc.scalar.activation(
                    dst[:, done : done + n], banks[b][:, off : off + n],
                    func, bias=fbias, scale=INV,
                )
                done += n

        def evict_and_gate(banks_s, stag, sbias, bc, gran):
            """Sigmoid-evict the first gemm's banks and fold in the bc gate.

            Emitted between the two gemm blocks of a jt so the m = bc*sig
            products overlap the second gemm — off the post-gemm critical
            path entirely.
            """
            out = []
            for q in range(Bs // gran):
                qs = slice(q * gran, (q + 1) * gran)
                s = apool.tile([P, gran], F16, name=stag, tag=stag)
                evict(banks_s, s, q, gran, AF.Sigmoid, sbias)
                m = dpool.tile([P, gran], F16, name="m", tag="m", bufs=4)
                nc.gpsimd.tensor_mul(m[:], bc[:, qs], s[:])
                out.append((qs, m))
            return out

        # PE warmup: dependency-free dummy matmuls fill the PE while the
        # cold-start DMA burst lands — ramping the PE power state so the
        # first real matmuls run at full clock. The first real matmul simply
        # queues behind at most one ~130ns dummy.
        warm = cpool.tile([P, 2, P], F8, name="warm")
        nc.vector.memset(warm[:], 0)
        wps = pspool.tile([P, ch], F32, name="wps", tag="ps")
        for _ in range(28):
            nc.tensor.matmul(
                wps[:, 0:P], lhsT=warm[:], rhs=warm[:],
                start=True, stop=True, perf_mode=DR,
            )

        # ---- phase B: d2 + y gates -> z_new ----
        prefetch_z(0)
        nc.gpsimd.partition_broadcast(bc2[:], bc2[0:1, :])
        nc.gpsimd.partition_broadcast(bc1[:], bc1[0:1, :])
        for jt in range(NJT):
            if jt + 2 < NJT:
                stream_w("wd2", jt + 2)
                stream_w("wy", jt + 2)
            elif jt == NJT - 2:
                stream_w("wd1", 0)
                stream_w("wg3", 0)
            if jt + 1 < NJT:
                prefetch_z(jt + 1)
            gran = ch if jt == NJT - 1 else half
            banks_d2 = gemm_block(wstream["wd2"].pop(jt), x8_sb, y8_sb)
            gates = evict_and_gate(banks_d2, "s2", bcol(0, jt), bc2, gran)
            banks_wy = gemm_block(wstream["wy"].pop(jt), x8_sb, y8_sb)

            zt = zt_tiles.pop(jt)
            for q, (qs, m) in enumerate(gates):
                tz = apool.tile([P, gran], F16, name="tz", tag="tz")
                evict(banks_wy, tz, q, gran, AF.Tanh, bcol(1, jt))
                d = dpool.tile([P, gran], F16, name="d", tag="d")
                nc.vector.tensor_sub(d[:], tz[:], zt[:, qs])
                p1 = dpool.tile([P, gran], F16, name="p1", tag="p1")
                nc.vector.tensor_mul(p1[:], m[:], d[:])
                znc = opool.tile([P, gran], F16, name="znc", tag="znc", bufs=3)
                nc.gpsimd.tensor_add(znc[:], zt[:, qs], p1[:])
                # rounding cast into the resident fp8 tile for GEMM3
                nc.vector.tensor_scalar_mul(zn8_sb[:, jt, qs], znc[:], SA)
                nc.sync.dma_start(zod[:, jt, qs], znc[:])

        # ---- phase C: d1 gate + (i_z + z_new @ W_z.T) -> y_new ----
        prefetch_y(0)
        for jt in range(NJT):
            if jt + 1 < NJT:
                prefetch_y(jt + 1)
                stream_w("wd1", jt + 1)
                stream_w("wg3", jt + 1)
            last = jt == NJT - 1
            gran = ch if jt >= NJT - 2 else half
            banks_d1 = gemm_block(wstream["wd1"].pop(jt), x8_sb, y8_sb)
            gates = evict_and_gate(banks_d1, "s1", bcol(2, jt), bc1, gran)
            banks_g3 = gemm_block(wstream["wg3"].pop(jt), x8_sb, zn8_sb, c_outer=last)

            yt = yt_tiles.pop(jt)
            for q, (qs, m) in enumerate(gates):
                u = apool.tile([P, gran], F16, name="u", tag="u")
                evict(banks_g3, u, q, gran, AF.Tanh, bcol(3, jt))
                d = dpool.tile([P, gran], F16, name="dy", tag="d")
                nc.vector.tensor_sub(d[:], u[:], yt[:, qs])
                p1 = dpool.tile([P, gran], F16, name="py1", tag="p1")
                nc.vector.tensor_mul(p1[:], m[:], d[:])
                yn = opool.tile([P, gran], F16, name="yn", tag="yn", bufs=3)
                nc.gpsimd.tensor_add(yn[:], yt[:, qs], p1[:])
                # late tiles: spread the output flush over parallel DMA rings
                oeng = [nc.sync, nc.scalar, nc.gpsimd][q % 3] if jt >= NJT - 3 else nc.sync
                oeng.dma_start(yod[:, jt, qs], yn[:])

    nc.compile()
    return nc


def _featmaj(a2d):
    """[B, F] -> [P, F//P, B] feature-major packing."""
    B, F = a2d.shape
    return np.ascontiguousarray(a2d.T.reshape(F // P, P, B).transpose(1, 0, 2))


def _packw(Wa, Wb):
    """Two [H, 1024] weight mats -> [NJT, P, 2*NKT, P] fp8 lhsT blocks.

    Block[jt, p, k, m] = Wcat[jt*P + m, k*P + p] * SW, Wcat = [Wa | Wb].
    """
    Wcat = np.concatenate([Wa, Wb], axis=1)
    R = Wcat.reshape(NJT, P, 2 * NKT, P).transpose(0, 3, 2, 1)
    return np.ascontiguousarray((R * SW).astype(E4))


def pack_host_inputs(x, y, z, dt, W_ih, b_ih, W_hh, b_hh, W_z, b_z, b_dt, n_cores):
    B = x.shape[0]
    Bs = B // n_cores

    x8 = (_featmaj(x) * SA).astype(E4)
    y8 = (_featmaj(y) * SA).astype(E4)
    z16 = _featmaj(z).astype(np.float16)
    y16 = _featmaj(y).astype(np.float16)
    dtrow = np.ascontiguousarray(dt.reshape(1, B))

    wd2 = _packw(W_ih[H : 2 * H], W_hh[H : 2 * H])
    wy = _packw(W_ih[3 * H : 4 * H], W_hh[2 * H : 3 * H])
    wd1 = _packw(W_ih[0:H], W_hh[0:H])
    wg3 = _packw(W_ih[2 * H : 3 * H], W_z)

    def bias_cols(bvec):
        return bvec.reshape(NJT, P).T  # [P, NJT]

    bdt_cols = np.zeros((P, 2), np.float32)
    bdt_cols[0, 0] = b_dt[0]
    bdt_cols[0, 1] = b_dt[1]
    biasP = np.ascontiguousarray(
        np.concatenate(
            [
                bias_cols(b_ih[H : 2 * H] + b_hh[H : 2 * H]),
                bias_cols(b_ih[3 * H : 4 * H] + b_hh[2 * H : 3 * H]),
                bias_cols(b_ih[0:H] + b_hh[0:H]),
                bias_cols(b_ih[2 * H : 3 * H] + b_z),
                bdt_cols,
            ],
            axis=1,
        ),
        dtype=np.float32,
    )

    in_maps = []
    for c in range(n_cores):
        cs = slice(c * Bs, (c + 1) * Bs)
        in_maps.append(
            {
                "x8": np.ascontiguousarray(x8[:, :, cs]),
                "y8": np.ascontiguousarray(y8[:, :, cs]),
                "z16": np.ascontiguousarray(z16[:, :, cs]),
                "y16": np.ascontiguousarray(y16[:, :, cs]),
                "dtr": np.ascontiguousarray(dtrow[:, cs]),
                "wd2": wd2,
                "wy": wy,
                "wd1": wd1,
                "wg3": wg3,
                "biasP": biasP,
            }
        )
    return in_maps


def _unpack_out(parts):
    """list of [P, NJT, Bs] core outputs -> [B, H] float32."""
    blocks = [
        np.asarray(a).astype(np.float32).transpose(2, 1, 0).reshape(a.shape[2], NJT * P)
        for a in parts
    ]
    return np.ascontiguousarray(np.concatenate(blocks, axis=0), dtype=np.float32)


def kernel(x, y, z, dt, W_ih, b_ih, W_hh, b_hh, W_z, b_z, W_dt, b_dt):
    x = np.asarray(x, np.float32)
    y = np.asarray(y, np.float32)
    z = np.asarray(z, np.float32)
    dt = np.asarray(dt, np.float32)
    W_ih = np.asarray(W_ih, np.float32)
    b_ih = np.asarray(b_ih, np.float32)
    W_hh = np.asarray(W_hh, np.float32)
    b_hh = np.asarray(b_hh, np.float32)
    W_z = np.asarray(W_z, np.float32)
    b_z = np.asarray(b_z, np.float32)
    W_dt = np.asarray(W_dt, np.float32)
    b_dt = np.asarray(b_dt, np.float32)

    B = x.shape[0]
    Bs = B // N_CORES

    in_maps = pack_host_inputs(
        x, y, z, dt, W_ih, b_ih, W_hh, b_hh, W_z, b_z, b_dt, N_CORES
    )
    nc = build_nc(Bs, wdt00=float(W_dt[0, 0]), wdt10=float(W_dt[1, 0]))

    trace = os.environ.get("LEM_TRACE", "0") == "1"
    tmpdir = os.environ.get("LEM_TMPDIR") or None
    res = run_bass_kernel_spmd(
        nc, in_maps, list(range(N_CORES)), trace=trace, tmpdir=tmpdir
    )
    global LAST_RESULTS
    LAST_RESULTS = res
    y_new = _unpack_out([r["y_newP"] for r in res.results])
    z_new = _unpack_out([r["z_newP"] for r in res.results])
    return (y_new, z_new)


# revision 26
# speedup vs baseline: 1.0079x; 1.0079x over previous
"""LEM cell (ODE2) Bass kernel for Trainium2, 8-core data-parallel, fp8 PE.

Math (per batch row b):
  ti = x @ W_ih.T + b_ih                  # [B, 4H]
  th = y @ W_hh.T + b_hh                  # [B, 3H]
  tdt = dt @ W_dt.T + b_dt                # [B, 2]
  ms_dt_bar = sig(tdt[:,0]) * sig(ti[:, :H]   + th[:, :H])
  ms_dt     = sig(tdt[:,1]) * sig(ti[:, H:2H] + th[:, H:2H])
  z_new = (1-ms_dt) * z + ms_dt * tanh(ti[:, 3H:] + th[:, 2H:3H])
  y_new = (1-ms_dt_bar) * y + ms_dt_bar * tanh(z_new @ W_z.T + b_z + i_z)
  returns (y_new, z_new)

Strategy: shard batch across 8 cores (2048 rows each). All on-chip tensors are
feature-major ([feature_tile=128 partitions, batch columns free]). The GEMMs
run as fp8e4 (e4m3) with the DoubleRow perf mode: each matmul contracts 256
features (two 128-tiles packed in the AP's dim1) at the 157 TF/s fp8 peak —
2x the fp32r rate of the fp32 baseline. Host pre-scales x/y by 16 and weights
by 4096 (powers of two; undone exactly by the activation scale 1/65536);
z_new is cast on-chip to fp8*16 for the third GEMM. The i+h / i_z+z@Wz sums
come free by accumulating both halves of each packed weight block into the
same PSUM bank. The whole pointwise pipeline (gates, blends, outputs) runs in
fp16 — the fp8 GEMM quantization error dominates, so fp16 rounding is
invisible (simulated end-to-end rel err 1.40e-2, gate 2e-2) — which halves
DVE/Pool SBUF traffic and the z/y/output DMA volume. The per-column dt-gate
products m = sig(tdt)*sig(gate) are computed while the second GEMM of each
feature tile streams, so only a 3-op chain (sub, mul, add) trails the last
matmul of a tile. The last feature tiles run at quarter granularity to
shorten the drain tail.
"""

import os
import sys

_REPO = "/opt/trn_rl_repo"
if _REPO not in sys.path:
    sys.path.insert(0, _REPO)

from contextlib import ExitStack

import numpy as np
import ml_dtypes

import concourse.bacc as bacc
import concourse.bass as bass
import concourse.tile as tile
from concourse import mybir
from concourse.bass_utils import run_bass_kernel_spmd

P = 128
F32 = mybir.dt.float32
F16 = mybir.dt.float16
F8 = mybir.dt.float8e4
E4 = ml_dtypes.float8_e4m3
AF = mybir.ActivationFunctionType
DR = mybir.MatmulPerfMode.DoubleRow

N_CORES = 8
H = 1024
K = 1024
NKT = K // P  # 8 contraction tiles per 1024-dim operand
NJT = H // P  # 8 output feature tiles
SA = 16.0     # activation fp8 pre-scale
SW = 4096.0   # weight fp8 pre-scale
INV = 1.0 / (SA * SW)

LAST_RESULTS = None  # BassKernelResults of the most recent kernel() call


def build_nc(Bs, wdt00, wdt10, ch=512, half=1024):
    NCH = Bs // ch    # PSUM banks per gemm block
    NH = Bs // half   # pointwise sub-tiles per jt

    nc = bacc.Bacc(trn_type="TRN2", target_bir_lowering=False)

    x8d = nc.declare_dram_parameter("x8", [P, NKT, Bs], F8, isOutput=False)
    y8d = nc.declare_dram_parameter("y8", [P, NKT, Bs], F8, isOutput=False)
    z16d = nc.declare_dram_parameter("z16", [P, NJT, Bs], F16, isOutput=False)
    y16d = nc.declare_dram_parameter("y16", [P, NJT, Bs], F16, isOutput=False)
    dtrd = nc.declare_dram_parameter("dtr", [1, Bs], F32, isOutput=False)
    Wd = {
        g: nc.declare_dram_parameter(g, [NJT, P, 2 * NKT, P], F8, isOutput=False)
        for g in ("wd2", "wy", "wd1", "wg3")
    }
    biasd = nc.declare_dram_parameter("biasP", [P, 4 * NJT + 2], F32, isOutput=False)
    zod = nc.declare_dram_parameter("z_newP", [P, NJT, Bs], F16, isOutput=True)
    yod = nc.declare_dram_parameter("y_newP", [P, NJT, Bs], F16, isOutput=True)

    with tile.TileContext(nc) as tc, ExitStack() as ctx:
        cpool = ctx.enter_context(tc.tile_pool(name="cpool", bufs=1))
        wstr = ctx.enter_context(tc.tile_pool(name="wstr", bufs=2))
        pspool = ctx.enter_context(tc.tile_pool(name="pspool", bufs=8, space="PSUM"))
        apool = ctx.enter_context(tc.tile_pool(name="apool", bufs=3))
        zpool = ctx.enter_context(tc.tile_pool(name="zpool", bufs=2))
        dpool = ctx.enter_context(tc.tile_pool(name="dpool", bufs=3))
        opool = ctx.enter_context(tc.tile_pool(name="opool", bufs=2))

        bias_sb = cpool.tile([P, 4 * NJT + 2], F32, name="bias_sb")
        dt_sb = cpool.tile([1, Bs], F32, name="dt_sb")

        def bcol(g, jt):
            i = g * NJT + jt
            return bias_sb[:, i : i + 1]

        # jt0/jt1 weights lead the scalar queue, halved so the x-contraction
        # half (used by the first 4 pair-steps) lands before the y half.
        wstream = {"wd2": {}, "wy": {}, "wd1": {}, "wg3": {}}

        def stream_w(g, jt, eng=None, split=False):
            t_ = wstr.tile([P, 2 * NKT, P], F8, name=g, tag=g)
            eng = eng or nc.sync
            if split:
                eng.dma_start(t_[:, 0:NKT, :], Wd[g][jt][:, 0:NKT, :])
                eng.dma_start(t_[:, NKT : 2 * NKT, :], Wd[g][jt][:, NKT : 2 * NKT, :])
            else:
                eng.dma_start(t_[:], Wd[g][jt])
            wstream[g][jt] = t_

        stream_w("wd2", 0, nc.scalar, split=True)
        stream_w("wy", 0, nc.scalar, split=True)

        # fp8 activations, resident for the whole kernel. x on sync, y on
        # gpsimd: separate queues so both stream from t=0, issue-ordered by
        # first PE use; column-halved so the first chunks land early during
        # the 8-core cold-start HBM burst.
        x8_sb = cpool.tile([P, NKT, Bs], F8, name="x8_sb")
        y8_sb = cpool.tile([P, NKT, Bs], F8, name="y8_sb")
        # h-major: all pairs of column-half 0 land first, matching the
        # col-split jt0/jt1 gemms that run on half the batch columns.
        hb = Bs // 2
        for h in range(2):
            for t in range(NKT // 2):
                hs = slice(h * hb, (h + 1) * hb)
                nc.sync.dma_start(x8_sb[:, 2 * t : 2 * t + 2, hs], x8d[:, 2 * t : 2 * t + 2, hs])
        for h in range(2):
            for t in range(NKT // 2):
                hs = slice(h * hb, (h + 1) * hb)
                nc.gpsimd.dma_start(y8_sb[:, 2 * t : 2 * t + 2, hs], y8d[:, 2 * t : 2 * t + 2, hs])
        zn8_sb = cpool.tile([P, NJT, Bs], F8, name="zn8_sb")

        nc.sync.dma_start(dt_sb[:], dtrd[0:1, :])
        nc.sync.dma_start(bias_sb[:], biasd[:, :])
        stream_w("wd2", 1, nc.scalar)
        stream_w("wy", 1, nc.scalar)

        # per-batch dt gates (fp16 broadcast tiles). The broadcasts are
        # emitted after the z16(0) prefetch: their sg-semaphore waits would
        # otherwise block the gpsimd queue head and delay that transfer.
        bc1 = cpool.tile([P, Bs], F16, name="bc1")
        bc2 = cpool.tile([P, Bs], F16, name="bc2")
        nc.scalar.activation(
            bc1[0:1, :], dt_sb[:], AF.Sigmoid,
            bias=bias_sb[0:1, 4 * NJT : 4 * NJT + 1], scale=wdt00,
        )
        nc.scalar.activation(
            bc2[0:1, :], dt_sb[:], AF.Sigmoid,
            bias=bias_sb[0:1, 4 * NJT + 1 : 4 * NJT + 2], scale=wdt10,
        )

        # streamed fp16 blend operands (z for phase B, y for phase C)
        zt_tiles, yt_tiles = {}, {}

        def prefetch_z(jt):
            zt = zpool.tile([P, Bs], F16, name="zt", tag="zt")
            nc.gpsimd.dma_start(zt[:], z16d[:, jt, :])
            zt_tiles[jt] = zt

        def prefetch_y(jt):
            yt = zpool.tile([P, Bs], F16, name="yt", tag="yt")
            nc.gpsimd.dma_start(yt[:], y16d[:, jt, :])
            yt_tiles[jt] = yt

        def gemm_block(w_sb, rhs_a, rhs_b, c_outer=False, col_split=1):
            """2048-deep contraction into NCH psum banks via DoubleRow pairs.

            Weight block dim1 holds 16 ktiles: [0:8] pair with rhs_a,
            [8:16] with rhs_b. c-inner keeps each weight pair loaded in the
            PE array for NCH consecutive matmuls; c_outer instead finishes
            each bank as early as possible (4x the weight loads — used only
            for the final block so its drain chain overlaps the gemm).
            """
            banks = [pspool.tile([P, ch], F32, name="ps", tag="ps") for _ in range(NCH)]

            def mm(t, c):
                if t < NKT // 2:
                    w_ap = w_sb[:, 2 * t : 2 * t + 2, :]
                    r_src, r0 = rhs_a, 2 * t
                else:
                    tb = t - NKT // 2
                    w_ap = w_sb[:, NKT + 2 * tb : NKT + 2 * tb + 2, :]
                    r_src, r0 = rhs_b, 2 * tb
                nc.tensor.matmul(
                    banks[c][:],
                    lhsT=w_ap,
                    rhs=r_src[:, r0 : r0 + 2, c * ch : (c + 1) * ch],
                    start=(t == 0),
                    stop=(t == NKT - 1),
                    perf_mode=DR,
                )

            if c_outer:
                for c in range(NCH):
                    for t in range(NKT):
                        mm(t, c)
            else:
                ns = NCH // col_split
                for s in range(col_split):
                    for t in range(NKT):
                        for c in range(s * ns, (s + 1) * ns):
                            mm(t, c)
            return banks

        def evict(banks, dst, q, gran, func, fbias):
            """ACT-evict psum bank range q*gran..(q+1)*gran into dst[0:gran]."""
            done = 0
            while done < gran:
                g0 = q * gran + done
                b, off = g0 // ch, g0 % ch
                n = min(ch - off, gran - done)
                nc.scalar.activation(
                    dst[:, done : done + n], banks[b][:, off : off + n],
                    func, bias=fbias, scale=INV,
                )
                done += n

        def evict_and_gate(banks_s, stag, sbias, bc, gran):
            """Sigmoid-evict the first gemm's banks and fold in the bc gate.

            Emitted between the two gemm blocks of a jt so the m = bc*sig
            products overlap the second gemm — off the post-gemm critical
            path entirely.
            """
            out = []
            for q in range(Bs // gran):
                qs = slice(q * gran, (q + 1) * gran)
                s = apool.tile([P, gran], F16, name=stag, tag=stag)
                evict(banks_s, s, q, gran, AF.Sigmoid, sbias)
                m = dpool.tile([P, gran], F16, name="m", tag="m", bufs=4)
                nc.gpsimd.tensor_mul(m[:], bc[:, qs], s[:])
                out.append((qs, m))
            return out

        # PE warmup: dependency-free dummy matmuls fill the PE while the
        # cold-start DMA burst lands — ramping the PE power state so the
        # first real matmuls run at full clock. The first real matmul simply
        # queues behind at most one ~130ns dummy.
        warm = cpool.tile([P, 2, P], F8, name="warm")
        nc.vector.memset(warm[:], 0)
        wps = pspool.tile([P, ch], F32, name="wps", tag="ps")
        for _ in range(28):
            nc.tensor.matmul(
                wps[:, 0:P], lhsT=warm[:], rhs=warm[:],
                start=True, stop=True, perf_mode=DR,
            )

        # ---- phase B: d2 + y gates -> z_new ----
        prefetch_z(0)
        nc.gpsimd.partition_broadcast(bc2[:], bc2[0:1, :])
        nc.gpsimd.partition_broadcast(bc1[:], bc1[0:1, :])
        for jt in range(NJT):
            if jt + 2 < NJT:
                stream_w("wd2", jt + 2)
                stream_w("wy", jt + 2)
            elif jt == NJT - 2:
                stream_w("wd1", 0)
                stream_w("wg3", 0)
            if jt + 1 < NJT:
                prefetch_z(jt + 1)
            gran = ch if jt == NJT - 1 else half
            csp = 2 if jt < 2 else 1
            banks_d2 = gemm_block(wstream["wd2"].pop(jt), x8_sb, y8_sb, col_split=csp)
            gates = evict_and_gate(banks_d2, "s2", bcol(0, jt), bc2, gran)
            banks_wy = gemm_block(wstream["wy"].pop(jt), x8_sb, y8_sb, col_split=csp)

            zt = zt_tiles.pop(jt)
            for q, (qs, m) in enumerate(gates):
                tz = apool.tile([P, gran], F16, name="tz", tag="tz")
                evict(banks_wy, tz, q, gran, AF.Tanh, bcol(1, jt))
                d = dpool.tile([P, gran], F16, name="d", tag="d")
                nc.vector.tensor_sub(d[:], tz[:], zt[:, qs])
                p1 = dpool.tile([P, gran], F16, name="p1", tag="p1")
                nc.vector.tensor_mul(p1[:], m[:], d[:])
                znc = opool.tile([P, gran], F16, name="znc", tag="znc", bufs=3)
                nc.gpsimd.tensor_add(znc[:], zt[:, qs], p1[:])
                # rounding cast into the resident fp8 tile for GEMM3
                nc.vector.tensor_scalar_mul(zn8_sb[:, jt, qs], znc[:], SA)
                nc.sync.dma_start(zod[:, jt, qs], znc[:])

        # ---- phase C: d1 gate + (i_z + z_new @ W_z.T) -> y_new ----
        prefetch_y(0)
        for jt in range(NJT):
            if jt + 1 < NJT:
                prefetch_y(jt + 1)
                stream_w("wd1", jt + 1)
                stream_w("wg3", jt + 1)
            last = jt == NJT - 1
            gran = ch if jt >= NJT - 2 else half
            banks_d1 = gemm_block(wstream["wd1"].pop(jt), x8_sb, y8_sb)
            gates = evict_and_gate(banks_d1, "s1", bcol(2, jt), bc1, gran)
            banks_g3 = gemm_block(wstream["wg3"].pop(jt), x8_sb, zn8_sb, c_outer=last)

            yt = yt_tiles.pop(jt)
            for q, (qs, m) in enumerate(gates):
                u = apool.tile([P, gran], F16, name="u", tag="u")
                evict(banks_g3, u, q, gran, AF.Tanh, bcol(3, jt))
                d = dpool.tile([P, gran], F16, name="dy", tag="d")
                nc.vector.tensor_sub(d[:], u[:], yt[:, qs])
                p1 = dpool.tile([P, gran], F16, name="py1", tag="p1")
                nc.vector.tensor_mul(p1[:], m[:], d[:])
                yn = opool.tile([P, gran], F16, name="yn", tag="yn", bufs=3)
                nc.gpsimd.tensor_add(yn[:], yt[:, qs], p1[:])
                nc.sync.dma_start(yod[:, jt, qs], yn[:])

    nc.compile()
    return nc


def _featmaj(a2d):
    """[B, F] -> [P, F//P, B] feature-major packing."""
    B, F = a2d.shape
    return np.ascontiguousarray(a2d.T.reshape(F // P, P, B).transpose(1, 0, 2))


def _packw(Wa, Wb):
    """Two [H, 1024] weight mats -> [NJT, P, 2*NKT, P] fp8 lhsT blocks.

    Block[jt, p, k, m] = Wcat[jt*P + m, k*P + p] * SW, Wcat = [Wa | Wb].
    """
    Wcat = np.concatenate([Wa, Wb], axis=1)
    R = Wcat.reshape(NJT, P, 2 * NKT, P).transpose(0, 3, 2, 1)
    return np.ascontiguousarray((R * SW).astype(E4))


def pack_host_inputs(x, y, z, dt, W_ih, b_ih, W_hh, b_hh, W_z, b_z, b_dt, n_cores):
    B = x.shape[0]
    Bs = B // n_cores

    x8 = (_featmaj(x) * SA).astype(E4)
    y8 = (_featmaj(y) * SA).astype(E4)
    z16 = _featmaj(z).astype(np.float16)
    y16 = _featmaj(y).astype(np.float16)
    dtrow = np.ascontiguousarray(dt.reshape(1, B))

    wd2 = _packw(W_ih[H : 2 * H], W_hh[H : 2 * H])
    wy = _packw(W_ih[3 * H : 4 * H], W_hh[2 * H : 3 * H])
    wd1 = _packw(W_ih[0:H], W_hh[0:H])
    wg3 = _packw(W_ih[2 * H : 3 * H], W_z)

    def bias_cols(bvec):
        return bvec.reshape(NJT, P).T  # [P, NJT]

    bdt_cols = np.zeros((P, 2), np.float32)
    bdt_cols[0, 0] = b_dt[0]
    bdt_cols[0, 1] = b_dt[1]
    biasP = np.ascontiguousarray(
        np.concatenate(
            [
                bias_cols(b_ih[H : 2 * H] + b_hh[H : 2 * H]),
                bias_cols(b_ih[3 * H : 4 * H] + b_hh[2 * H : 3 * H]),
                bias_cols(b_ih[0:H] + b_hh[0:H]),
                bias_cols(b_ih[2 * H : 3 * H] + b_z),
                bdt_cols,
            ],
            axis=1,
        ),
        dtype=np.float32,
    )

    in_maps = []
    for c in range(n_cores):
        cs = slice(c * Bs, (c + 1) * Bs)
        in_maps.append(
            {
                "x8": np.ascontiguousarray(x8[:, :, cs]),
                "y8": np.ascontiguousarray(y8[:, :, cs]),
                "z16": np.ascontiguousarray(z16[:, :, cs]),
                "y16": np.ascontiguousarray(y16[:, :, cs]),
                "dtr": np.ascontiguousarray(dtrow[:, cs]),
                "wd2": wd2,
                "wy": wy,
                "wd1": wd1,
                "wg3": wg3,
                "biasP": biasP,
            }
        )
    return in_maps


def _unpack_out(parts):
    """list of [P, NJT, Bs] core outputs -> [B, H] float32."""
    blocks = [
        np.asarray(a).astype(np.float32).transpose(2, 1, 0).reshape(a.shape[2], NJT * P)
        for a in parts
    ]
    return np.ascontiguousarray(np.concatenate(blocks, axis=0), dtype=np.float32)


def kernel(x, y, z, dt, W_ih, b_ih, W_hh, b_hh, W_z, b_z, W_dt, b_dt):
    x = np.asarray(x, np.float32)
    y = np.asarray(y, np.float32)
    z = np.asarray(z, np.float32)
    dt = np.asarray(dt, np.float32)
    W_ih = np.asarray(W_ih, np.float32)
    b_ih = np.asarray(b_ih, np.float32)
    W_hh = np.asarray(W_hh, np.float32)
    b_hh = np.asarray(b_hh, np.float32)
    W_z = np.asarray(W_z, np.float32)
    b_z = np.asarray(b_z, np.float32)
    W_dt = np.asarray(W_dt, np.float32)
    b_dt = np.asarray(b_dt, np.float32)

    B = x.shape[0]
    Bs = B // N_CORES

    in_maps = pack_host_inputs(
        x, y, z, dt, W_ih, b_ih, W_hh, b_hh, W_z, b_z, b_dt, N_CORES
    )
    nc = build_nc(Bs, wdt00=float(W_dt[0, 0]), wdt10=float(W_dt[1, 0]))

    trace = os.environ.get("LEM_TRACE", "0") == "1"
    tmpdir = os.environ.get("LEM_TMPDIR") or None
    res = run_bass_kernel_spmd(
        nc, in_maps, list(range(N_CORES)), trace=trace, tmpdir=tmpdir
    )
    global LAST_RESULTS
    LAST_RESULTS = res
    y_new = _unpack_out([r["y_newP"] for r in res.results])
    z_new = _unpack_out([r["z_newP"] for r in res.results])
    return (y_new, z_new)


# revision 27
# speedup vs baseline: 1.0172x; 1.0093x over previous
"""LEM cell (ODE2) Bass kernel for Trainium2, 8-core data-parallel, fp8 PE.

Math (per batch row b):
  ti = x @ W_ih.T + b_ih                  # [B, 4H]
  th = y @ W_hh.T + b_hh                  # [B, 3H]
  tdt = dt @ W_dt.T + b_dt                # [B, 2]
  ms_dt_bar = sig(tdt[:,0]) * sig(ti[:, :H]   + th[:, :H])
  ms_dt     = sig(tdt[:,1]) * sig(ti[:, H:2H] + th[:, H:2H])
  z_new = (1-ms_dt) * z + ms_dt * tanh(ti[:, 3H:] + th[:, 2H:3H])
  y_new = (1-ms_dt_bar) * y + ms_dt_bar * tanh(z_new @ W_z.T + b_z + i_z)
  returns (y_new, z_new)

Strategy: shard batch across 8 cores (2048 rows each). All on-chip tensors are
feature-major ([feature_tile=128 partitions, batch columns free]). The GEMMs
run as fp8e4 (e4m3) with the DoubleRow perf mode: each matmul contracts 256
features (two 128-tiles packed in the AP's dim1) at the 157 TF/s fp8 peak —
2x the fp32r rate of the fp32 baseline. Host pre-scales x/y by 16 and weights
by 4096 (powers of two; undone exactly by the activation scale 1/65536);
z_new is cast on-chip to fp8*16 for the third GEMM. The i+h / i_z+z@Wz sums
come free by accumulating both halves of each packed weight block into the
same PSUM bank. The whole pointwise pipeline (gates, blends, outputs) runs in
fp16 — the fp8 GEMM quantization error dominates, so fp16 rounding is
invisible (simulated end-to-end rel err 1.40e-2, gate 2e-2) — which halves
DVE/Pool SBUF traffic and the z/y/output DMA volume. The per-column dt-gate
products m = sig(tdt)*sig(gate) are computed while the second GEMM of each
feature tile streams, so only a 3-op chain (sub, mul, add) trails the last
matmul of a tile. The last feature tiles run at quarter granularity to
shorten the drain tail.
"""

import os
import sys

_REPO = "/opt/trn_rl_repo"
if _REPO not in sys.path:
    sys.path.insert(0, _REPO)

from contextlib import ExitStack

import numpy as np
import ml_dtypes

import concourse.bacc as bacc
import concourse.bass as bass
import concourse.tile as tile
from concourse import mybir
from concourse.bass_utils import run_bass_kernel_spmd

P = 128
F32 = mybir.dt.float32
F16 = mybir.dt.float16
F8 = mybir.dt.float8e4
E4 = ml_dtypes.float8_e4m3
AF = mybir.ActivationFunctionType
DR = mybir.MatmulPerfMode.DoubleRow

N_CORES = 8
H = 1024
K = 1024
NKT = K // P  # 8 contraction tiles per 1024-dim operand
NJT = H // P  # 8 output feature tiles
SA = 16.0     # activation fp8 pre-scale
SW = 4096.0   # weight fp8 pre-scale
INV = 1.0 / (SA * SW)

LAST_RESULTS = None  # BassKernelResults of the most recent kernel() call


def build_nc(Bs, wdt00, wdt10, ch=512, half=1024):
    NCH = Bs // ch    # PSUM banks per gemm block
    NH = Bs // half   # pointwise sub-tiles per jt

    nc = bacc.Bacc(trn_type="TRN2", target_bir_lowering=False)

    x8d = nc.declare_dram_parameter("x8", [P, NKT, Bs], F8, isOutput=False)
    y8d = nc.declare_dram_parameter("y8", [P, NKT, Bs], F8, isOutput=False)
    z16d = nc.declare_dram_parameter("z16", [P, NJT, Bs], F16, isOutput=False)
    y16d = nc.declare_dram_parameter("y16", [P, NJT, Bs], F16, isOutput=False)
    dtrd = nc.declare_dram_parameter("dtr", [1, Bs], F32, isOutput=False)
    Wd = {
        g: nc.declare_dram_parameter(g, [NJT, P, 2 * NKT, P], F8, isOutput=False)
        for g in ("wd2", "wy", "wd1", "wg3")
    }
    biasd = nc.declare_dram_parameter("biasP", [P, 4 * NJT + 2], F32, isOutput=False)
    zod = nc.declare_dram_parameter("z_newP", [P, NJT, Bs], F16, isOutput=True)
    yod = nc.declare_dram_parameter("y_newP", [P, NJT, Bs], F16, isOutput=True)

    with tile.TileContext(nc) as tc, ExitStack() as ctx:
        cpool = ctx.enter_context(tc.tile_pool(name="cpool", bufs=1))
        wstr = ctx.enter_context(tc.tile_pool(name="wstr", bufs=2))
        pspool = ctx.enter_context(tc.tile_pool(name="pspool", bufs=8, space="PSUM"))
        apool = ctx.enter_context(tc.tile_pool(name="apool", bufs=3))
        zpool = ctx.enter_context(tc.tile_pool(name="zpool", bufs=2))
        dpool = ctx.enter_context(tc.tile_pool(name="dpool", bufs=3))
        opool = ctx.enter_context(tc.tile_pool(name="opool", bufs=2))

        bias_sb = cpool.tile([P, 4 * NJT + 2], F32, name="bias_sb")
        dt_sb = cpool.tile([1, Bs], F32, name="dt_sb")

        def bcol(g, jt):
            i = g * NJT + jt
            return bias_sb[:, i : i + 1]

        # jt0/jt1 weights lead the scalar queue, halved so the x-contraction
        # half (used by the first 4 pair-steps) lands before the y half.
        wstream = {"wd2": {}, "wy": {}, "wd1": {}, "wg3": {}}

        def stream_w(g, jt, eng=None, split=False):
            t_ = wstr.tile([P, 2 * NKT, P], F8, name=g, tag=g)
            eng = eng or nc.sync
            if split:
                eng.dma_start(t_[:, 0:NKT, :], Wd[g][jt][:, 0:NKT, :])
                eng.dma_start(t_[:, NKT : 2 * NKT, :], Wd[g][jt][:, NKT : 2 * NKT, :])
            else:
                eng.dma_start(t_[:], Wd[g][jt])
            wstream[g][jt] = t_

        stream_w("wd2", 0, nc.scalar, split=True)
        stream_w("wy", 0, nc.scalar, split=True)

        # fp8 activations, resident for the whole kernel. x on sync, y on
        # gpsimd: separate queues so both stream from t=0, issue-ordered by
        # first PE use; column-halved so the first chunks land early during
        # the 8-core cold-start HBM burst.
        x8_sb = cpool.tile([P, NKT, Bs], F8, name="x8_sb")
        y8_sb = cpool.tile([P, NKT, Bs], F8, name="y8_sb")
        # h-major and ring-balanced: column-half 0 (all pairs) lands first
        # for the col-split jt0/jt1 gemms; the four half-tensors ride four
        # ring assignments so no ring carries more than ~2 MB of the
        # cold-start burst. Need order: x-h0, y-h0 (sub-block 0), then
        # x-h1/y-h1 (sub-block 1) in parallel on scalar/sync.
        hb = Bs // 2
        h0, h1 = slice(0, hb), slice(hb, Bs)
        for t in range(NKT // 2):
            nc.sync.dma_start(x8_sb[:, 2 * t : 2 * t + 2, h0], x8d[:, 2 * t : 2 * t + 2, h0])
        for t in range(NKT // 2):
            nc.gpsimd.dma_start(y8_sb[:, 2 * t : 2 * t + 2, h0], y8d[:, 2 * t : 2 * t + 2, h0])
        for t in range(NKT // 2):
            nc.scalar.dma_start(x8_sb[:, 2 * t : 2 * t + 2, h1], x8d[:, 2 * t : 2 * t + 2, h1])
        for t in range(NKT // 2):
            nc.sync.dma_start(y8_sb[:, 2 * t : 2 * t + 2, h1], y8d[:, 2 * t : 2 * t + 2, h1])
        zn8_sb = cpool.tile([P, NJT, Bs], F8, name="zn8_sb")

        nc.sync.dma_start(dt_sb[:], dtrd[0:1, :])
        nc.sync.dma_start(bias_sb[:], biasd[:, :])
        stream_w("wd2", 1, nc.scalar)
        stream_w("wy", 1, nc.scalar)

        # per-batch dt gates (fp16 broadcast tiles). The broadcasts are
        # emitted after the z16(0) prefetch: their sg-semaphore waits would
        # otherwise block the gpsimd queue head and delay that transfer.
        bc1 = cpool.tile([P, Bs], F16, name="bc1")
        bc2 = cpool.tile([P, Bs], F16, name="bc2")
        nc.scalar.activation(
            bc1[0:1, :], dt_sb[:], AF.Sigmoid,
            bias=bias_sb[0:1, 4 * NJT : 4 * NJT + 1], scale=wdt00,
        )
        nc.scalar.activation(
            bc2[0:1, :], dt_sb[:], AF.Sigmoid,
            bias=bias_sb[0:1, 4 * NJT + 1 : 4 * NJT + 2], scale=wdt10,
        )

        # streamed fp16 blend operands (z for phase B, y for phase C)
        zt_tiles, yt_tiles = {}, {}

        def prefetch_z(jt):
            zt = zpool.tile([P, Bs], F16, name="zt", tag="zt")
            nc.gpsimd.dma_start(zt[:], z16d[:, jt, :])
            zt_tiles[jt] = zt

        def prefetch_y(jt):
            yt = zpool.tile([P, Bs], F16, name="yt", tag="yt")
            nc.gpsimd.dma_start(yt[:], y16d[:, jt, :])
            yt_tiles[jt] = yt

        def gemm_block(w_sb, rhs_a, rhs_b, c_outer=False, col_split=1):
            """2048-deep contraction into NCH psum banks via DoubleRow pairs.

            Weight block dim1 holds 16 ktiles: [0:8] pair with rhs_a,
            [8:16] with rhs_b. c-inner keeps each weight pair loaded in the
            PE array for NCH consecutive matmuls; c_outer instead finishes
            each bank as early as possible (4x the weight loads — used only
            for the final block so its drain chain overlaps the gemm).
            """
            banks = [pspool.tile([P, ch], F32, name="ps", tag="ps") for _ in range(NCH)]

            def mm(t, c):
                if t < NKT // 2:
                    w_ap = w_sb[:, 2 * t : 2 * t + 2, :]
                    r_src, r0 = rhs_a, 2 * t
                else:
                    tb = t - NKT // 2
                    w_ap = w_sb[:, NKT + 2 * tb : NKT + 2 * tb + 2, :]
                    r_src, r0 = rhs_b, 2 * tb
                nc.tensor.matmul(
                    banks[c][:],
                    lhsT=w_ap,
                    rhs=r_src[:, r0 : r0 + 2, c * ch : (c + 1) * ch],
                    start=(t == 0),
                    stop=(t == NKT - 1),
                    perf_mode=DR,
                )

            if c_outer:
                for c in range(NCH):
                    for t in range(NKT):
                        mm(t, c)
            else:
                ns = NCH // col_split
                for s in range(col_split):
                    for t in range(NKT):
                        for c in range(s * ns, (s + 1) * ns):
                            mm(t, c)
            return banks

        def evict(banks, dst, q, gran, func, fbias):
            """ACT-evict psum bank range q*gran..(q+1)*gran into dst[0:gran]."""
            done = 0
            while done < gran:
                g0 = q * gran + done
                b, off = g0 // ch, g0 % ch
                n = min(ch - off, gran - done)
                nc.scalar.activation(
                    dst[:, done : done + n], banks[b][:, off : off + n],
                    func, bias=fbias, scale=INV,
                )
                done += n

        def evict_and_gate(banks_s, stag, sbias, bc, gran):
            """Sigmoid-evict the first gemm's banks and fold in the bc gate.

            Emitted between the two gemm blocks of a jt so the m = bc*sig
            products overlap the second gemm — off the post-gemm critical
            path entirely.
            """
            out = []
            for q in range(Bs // gran):
                qs = slice(q * gran, (q + 1) * gran)
                s = apool.tile([P, gran], F16, name=stag, tag=stag)
                evict(banks_s, s, q, gran, AF.Sigmoid, sbias)
                m = dpool.tile([P, gran], F16, name="m", tag="m", bufs=4)
                nc.gpsimd.tensor_mul(m[:], bc[:, qs], s[:])
                out.append((qs, m))
            return out

        # PE warmup: dependency-free dummy matmuls fill the PE while the
        # cold-start DMA burst lands — ramping the PE power state so the
        # first real matmuls run at full clock. The first real matmul simply
        # queues behind at most one ~130ns dummy.
        warm = cpool.tile([P, 2, P], F8, name="warm")
        nc.vector.memset(warm[:], 0)
        wps = pspool.tile([P, ch], F32, name="wps", tag="ps")
        for _ in range(28):
            nc.tensor.matmul(
                wps[:, 0:P], lhsT=warm[:], rhs=warm[:],
                start=True, stop=True, perf_mode=DR,
            )

        # ---- phase B: d2 + y gates -> z_new ----
        prefetch_z(0)
        nc.gpsimd.partition_broadcast(bc2[:], bc2[0:1, :])
        nc.gpsimd.partition_broadcast(bc1[:], bc1[0:1, :])
        for jt in range(NJT):
            if jt + 2 < NJT:
                stream_w("wd2", jt + 2)
                stream_w("wy", jt + 2)
            elif jt == NJT - 2:
                stream_w("wd1", 0)
                stream_w("wg3", 0)
            if jt + 1 < NJT:
                prefetch_z(jt + 1)
            gran = ch if jt == NJT - 1 else half
            csp = 2 if jt < 2 else 1
            banks_d2 = gemm_block(wstream["wd2"].pop(jt), x8_sb, y8_sb, col_split=csp)
            gates = evict_and_gate(banks_d2, "s2", bcol(0, jt), bc2, gran)
            banks_wy = gemm_block(wstream["wy"].pop(jt), x8_sb, y8_sb, col_split=csp)

            zt = zt_tiles.pop(jt)
            for q, (qs, m) in enumerate(gates):
                tz = apool.tile([P, gran], F16, name="tz", tag="tz")
                evict(banks_wy, tz, q, gran, AF.Tanh, bcol(1, jt))
                d = dpool.tile([P, gran], F16, name="d", tag="d")
                nc.vector.tensor_sub(d[:], tz[:], zt[:, qs])
                p1 = dpool.tile([P, gran], F16, name="p1", tag="p1")
                nc.vector.tensor_mul(p1[:], m[:], d[:])
                znc = opool.tile([P, gran], F16, name="znc", tag="znc", bufs=3)
                nc.gpsimd.tensor_add(znc[:], zt[:, qs], p1[:])
                # rounding cast into the resident fp8 tile for GEMM3
                nc.vector.tensor_scalar_mul(zn8_sb[:, jt, qs], znc[:], SA)
                nc.sync.dma_start(zod[:, jt, qs], znc[:])

        # ---- phase C: d1 gate + (i_z + z_new @ W_z.T) -> y_new ----
        prefetch_y(0)
        for jt in range(NJT):
            if jt + 1 < NJT:
                prefetch_y(jt + 1)
                stream_w("wd1", jt + 1)
                stream_w("wg3", jt + 1)
            last = jt == NJT - 1
            gran = ch if jt >= NJT - 2 else half
            banks_d1 = gemm_block(wstream["wd1"].pop(jt), x8_sb, y8_sb)
            gates = evict_and_gate(banks_d1, "s1", bcol(2, jt), bc1, gran)
            banks_g3 = gemm_block(wstream["wg3"].pop(jt), x8_sb, zn8_sb, c_outer=last)

            yt = yt_tiles.pop(jt)
            for q, (qs, m) in enumerate(gates):
                u = apool.tile([P, gran], F16, name="u", tag="u")
                evict(banks_g3, u, q, gran, AF.Tanh, bcol(3, jt))
                d = dpool.tile([P, gran], F16, name="dy", tag="d")
                nc.vector.tensor_sub(d[:], u[:], yt[:, qs])
                p1 = dpool.tile([P, gran], F16, name="py1", tag="p1")
                nc.vector.tensor_mul(p1[:], m[:], d[:])
                yn = opool.tile([P, gran], F16, name="yn", tag="yn", bufs=3)
                nc.gpsimd.tensor_add(yn[:], yt[:, qs], p1[:])
                nc.sync.dma_start(yod[:, jt, qs], yn[:])

    nc.compile()
    return nc


def _featmaj(a2d):
    """[B, F] -> [P, F//P, B] feature-major packing."""
    B, F = a2d.shape
    return np.ascontiguousarray(a2d.T.reshape(F // P, P, B).transpose(1, 0, 2))


def _packw(Wa, Wb):
    """Two [H, 1024] weight mats -> [NJT, P, 2*NKT, P] fp8 lhsT blocks.

    Block[jt, p, k, m] = Wcat[jt*P + m, k*P + p] * SW, Wcat = [Wa | Wb].
    """
    Wcat = np.concatenate([Wa, Wb], axis=1)
    R = Wcat.reshape(NJT, P, 2 * NKT, P).transpose(0, 3, 2, 1)
    return np.ascontiguousarray((R * SW).astype(E4))


def pack_host_inputs(x, y, z, dt, W_ih, b_ih, W_hh, b_hh, W_z, b_z, b_dt, n_cores):
    B = x.shape[0]
    Bs = B // n_cores

    x8 = (_featmaj(x) * SA).astype(E4)
    y8 = (_featmaj(y) * SA).astype(E4)
    z16 = _featmaj(z).astype(np.float16)
    y16 = _featmaj(y).astype(np.float16)
    dtrow = np.ascontiguousarray(dt.reshape(1, B))

    wd2 = _packw(W_ih[H : 2 * H], W_hh[H : 2 * H])
    wy = _packw(W_ih[3 * H : 4 * H], W_hh[2 * H : 3 * H])
    wd1 = _packw(W_ih[0:H], W_hh[0:H])
    wg3 = _packw(W_ih[2 * H : 3 * H], W_z)

    def bias_cols(bvec):
        return bvec.reshape(NJT, P).T  # [P, NJT]

    bdt_cols = np.zeros((P, 2), np.float32)
    bdt_cols[0, 0] = b_dt[0]
    bdt_cols[0, 1] = b_dt[1]
    biasP = np.ascontiguousarray(
        np.concatenate(
            [
                bias_cols(b_ih[H : 2 * H] + b_hh[H : 2 * H]),
                bias_cols(b_ih[3 * H : 4 * H] + b_hh[2 * H : 3 * H]),
                bias_cols(b_ih[0:H] + b_hh[0:H]),
                bias_cols(b_ih[2 * H : 3 * H] + b_z),
                bdt_cols,
            ],
            axis=1,
        ),
        dtype=np.float32,
    )

    in_maps = []
    for c in range(n_cores):
        cs = slice(c * Bs, (c + 1) * Bs)
        in_maps.append(
            {
                "x8": np.ascontiguousarray(x8[:, :, cs]),
                "y8": np.ascontiguousarray(y8[:, :, cs]),
                "z16": np.ascontiguousarray(z16[:, :, cs]),
                "y16": np.ascontiguousarray(y16[:, :, cs]),
                "dtr": np.ascontiguousarray(dtrow[:, cs]),
                "wd2": wd2,
                "wy": wy,
                "wd1": wd1,
                "wg3": wg3,
                "biasP": biasP,
            }
        )
    return in_maps


def _unpack_out(parts):
    """list of [P, NJT, Bs] core outputs -> [B, H] float32."""
    blocks = [
        np.asarray(a).astype(np.float32).transpose(2, 1, 0).reshape(a.shape[2], NJT * P)
        for a in parts
    ]
    return np.ascontiguousarray(np.concatenate(blocks, axis=0), dtype=np.float32)


def kernel(x, y, z, dt, W_ih, b_ih, W_hh, b_hh, W_z, b_z, W_dt, b_dt):
    x = np.asarray(x, np.float32)
    y = np.asarray(y, np.float32)
    z = np.asarray(z, np.float32)
    dt = np.asarray(dt, np.float32)
    W_ih = np.asarray(W_ih, np.float32)
    b_ih = np.asarray(b_ih, np.float32)
    W_hh = np.asarray(W_hh, np.float32)
    b_hh = np.asarray(b_hh, np.float32)
    W_z = np.asarray(W_z, np.float32)
    b_z = np.asarray(b_z, np.float32)
    W_dt = np.asarray(W_dt, np.float32)
    b_dt = np.asarray(b_dt, np.float32)

    B = x.shape[0]
    Bs = B // N_CORES

    in_maps = pack_host_inputs(
        x, y, z, dt, W_ih, b_ih, W_hh, b_hh, W_z, b_z, b_dt, N_CORES
    )
    nc = build_nc(Bs, wdt00=float(W_dt[0, 0]), wdt10=float(W_dt[1, 0]))

    trace = os.environ.get("LEM_TRACE", "0") == "1"
    tmpdir = os.environ.get("LEM_TMPDIR") or None
    res = run_bass_kernel_spmd(
        nc, in_maps, list(range(N_CORES)), trace=trace, tmpdir=tmpdir
    )
    global LAST_RESULTS
    LAST_RESULTS = res
    y_new = _unpack_out([r["y_newP"] for r in res.results])
    z_new = _unpack_out([r["z_newP"] for r in res.results])
    return (y_new, z_new)
